# revision 10
# baseline (speedup 1.0000x reference)
"""Trainium2 Bass kernel for nn_Knowledge_Manager (moe_routing).

Sharding: data-parallel over batch (32 batches -> 4 per core) for everything
except the 32000-vocab output projection, which is tensor-parallel over the
vocab dim (4000 per core) with a sharded log_softmax (AllGather of the
selected knowledge vector + AllReduce of the per-core sum-of-exp).

Self-contained: builds the Bass program, shards the full inputs, runs on the
8 NeuronCores via run_bass_kernel_spmd, and reassembles full outputs.
"""
import numpy as np
import orjson

import concourse.bass as bass
import concourse.tile as tile
from concourse import mybir
from concourse.bass_utils import run_bass_kernel_spmd
from concourse.masks import make_identity

F32 = mybir.dt.float32
I32 = mybir.dt.int32
AX = mybir.AxisListType.X
AF = mybir.ActivationFunctionType
ALU = mybir.AluOpType

B, N, XS, YS, KS, D, V = 32, 32, 256, 256, 128, 1024, 32000
NC = 8            # cores
BC = B // NC      # batches per core (4)
VC = V // NC      # vocab slice per core (4000)
TAU = 1e-4

# knobs for the dev harness (test.py); the grading path leaves them alone
TRACE = False
TRACE_KW = {}
LAST_RESULTS = None

_ctr = [0]


def _fix_bir_json(data: bytes) -> bytes:
    # This container's walrus encodes at most ONE semaphore wait per
    # instruction; split extra waits onto prefix same-engine NoOps.
    j = orjson.loads(data)
    changed = False
    for f in j.get("functions", []):
        for b in f.get("blocks", []):
            out = []
            for ins in b.get("instructions", []):
                si = ins.get("sync_info")
                waits = si.get("on_wait") if si else None
                if waits and len(waits) > 1:
                    changed = True
                    for w in waits[:-1]:
                        _ctr[0] += 1
                        out.append({
                            "debug": ins.get("debug"),
                            "engine": ins["engine"],
                            "ins": [],
                            "name": f"I-wfx-{_ctr[0]}",
                            "opcode": "NoOp",
                            "outs": [],
                            "text_hint": "waitfix",
                            "sync_info": {"on_update": [], "on_wait": [w]},
                        })
                    si["on_wait"] = [waits[-1]]
                out.append(ins)
            b["instructions"] = out
    return orjson.dumps(j) if changed else data


class _FixedBass(bass.Bass):
    def to_json_bytes(self) -> bytes:
        return _fix_bir_json(super().to_json_bytes())


def _bcast_ap(handle_ap, nparts, nfree, offset=0):
    """(1, nfree) DRAM source broadcast to nparts partitions."""
    return bass.AP(tensor=handle_ap.tensor, offset=offset,
                   ap=[[0, nparts], [1, nfree]])


def build_program():
    nc = _FixedBass()

    # ---- per-core inputs ----
    u = nc.dram_tensor("u", [XS, BC, D], F32, kind="ExternalInput")
    r = nc.dram_tensor("r", [YS, BC, D], F32, kind="ExternalInput")
    kh = nc.dram_tensor("kh", [N, KS, BC, D], F32, kind="ExternalInput")
    ubd = nc.dram_tensor("ubd", [128, 2, 8, BC], F32, kind="ExternalInput")
    kbd = nc.dram_tensor("kbd", [128, BC, 32, 32], F32, kind="ExternalInput")
    wk_nsb = nc.dram_tensor("wk_nsb", [N, KS, BC], F32, kind="ExternalInput")
    gum = nc.dram_tensor("gum", [BC, N], F32, kind="ExternalInput")
    w1t = nc.dram_tensor("w1t", [2 * D, D], F32, kind="ExternalInput")
    b1r = nc.dram_tensor("b1r", [1, D], F32, kind="ExternalInput")
    w2t = nc.dram_tensor("w2t", [D, VC], F32, kind="ExternalInput")
    b2r = nc.dram_tensor("b2r", [1, VC], F32, kind="ExternalInput")
    iota128 = nc.dram_tensor("iota128", [128, 1], F32, kind="ExternalInput")
    iota32 = nc.dram_tensor("iota32", [1, N], F32, kind="ExternalInput")

    # ---- per-core outputs ----
    prior_o = nc.dram_tensor("prior_o", [BC, N], F32, kind="ExternalOutput")
    post_o = nc.dram_tensor("post_o", [BC, N], F32, kind="ExternalOutput")
    kidx_o = nc.dram_tensor("kidx_o", [BC, N], F32, kind="ExternalOutput")
    sel_o = nc.dram_tensor("sel_o", [BC, KS, D], F32, kind="ExternalOutput")
    vocab_o = nc.dram_tensor("vocab_o", [B, VC], F32, kind="ExternalOutput")

    kh_flat = kh.rearrange("n s b d -> (n s) (b d)")
    wk_flat = wk_nsb.rearrange("n s b -> (n s) b")

    with tile.TileContext(nc) as tc:
        with (
            tc.tile_pool(name="persist", bufs=1) as pp,
            tc.tile_pool(name="stream", bufs=4) as sp,
            tc.tile_pool(name="wstream", bufs=3) as wp,
            tc.tile_pool(name="small", bufs=2) as mp,
            tc.tile_pool(name="selp", bufs=2) as selp,
            tc.tile_pool(name="big1", bufs=1) as bp,
            tc.tile_pool(name="dram", bufs=1, space="DRAM") as dp,
            tc.tile_pool(name="ps_a", bufs=1, space="PSUM") as ps_a,
            tc.tile_pool(name="ps_s", bufs=2, space="PSUM") as ps_s,
            tc.tile_pool(name="ps_v", bufs=1, space="PSUM") as ps_v,
        ):
            ident = pp.tile([128, 128], F32, tag="ident")
            make_identity(nc, ident)

            # persistent small inputs
            ubd_s = pp.tile([128, 2, 8, BC], F32, tag="ubd_s")
            nc.sync.dma_start(out=ubd_s[:], in_=ubd[:])
            kbd_s = pp.tile([128, BC, 32, 32], F32, tag="kbd_s")
            nc.sync.dma_start(out=kbd_s[:], in_=kbd[:])
            iota128_s = pp.tile([128, 1], F32, tag="iota128_s")
            nc.sync.dma_start(out=iota128_s[:], in_=iota128[:])

            # ============ mean_X / mean_Y ============
            # masked mean over sequence via block-diagonal packed matvecs:
            # psum rows (BC, D) accumulate 8 steps (2 s-chunks x 4 arrangements)
            mxy_rows = []
            for src_i, src in enumerate((u, r)):
                pxy = ps_a.tile([BC, D], F32, tag="acc")
                for sc in range(2):
                    for a in range(4):
                        s0 = sc * 128 + 32 * a
                        rhs = sp.tile([128, D], F32, tag="rhs")
                        nc.sync.dma_start(
                            out=rhs[:],
                            in_=src[s0:s0 + 32, :, :].rearrange("s b d -> b s d"))
                        step = sc * 4 + a
                        for h in range(2):
                            nc.tensor.matmul(
                                pxy[:, 512 * h:512 * (h + 1)],
                                lhsT=ubd_s[:, src_i, step, :],
                                rhs=rhs[:, 512 * h:512 * (h + 1)],
                                start=(step == 0), stop=(step == 7))
                rows = pp.tile([BC, D], F32, tag=f"mxy{src_i}")
                nc.vector.tensor_copy(out=rows[:], in_=pxy[:])
                mxy_rows.append(rows)

            # transpose to (d, b) column layout: (128, src, chunk, b)
            mxy_db = pp.tile([128, 2, 8, BC], F32, tag="mxy_db")
            for src_i in range(2):
                for c in range(8):
                    ptr = ps_s.tile([128, 32], F32, tag="tr")
                    nc.tensor.transpose(
                        out=ptr[:, :BC], in_=mxy_rows[src_i][:, 128 * c:128 * (c + 1)],
                        identity=ident[:BC, :BC])
                    nc.vector.tensor_copy(out=mxy_db[:, src_i, c, :], in_=ptr[:, :BC])

            # ============ mean_K ============
            # per-batch (32 n, D) tiles; 32 accumulating block-diagonal matmuls
            meank_b = [pp.tile([N, D], F32, tag=f"meank_b{b}", name=f"meank_b{b}")
                       for b in range(BC)]
            for b in range(BC):
                pmk = ps_a.tile([N, D], F32, tag="acc")
                for a in range(32):  # 4-row s-arrangements
                    rhs = sp.tile([128, D], F32, tag="rhs")
                    nc.sync.dma_start(
                        out=rhs[:],
                        in_=kh[:, 4 * a:4 * (a + 1), b, :])
                    for h in range(2):
                        nc.tensor.matmul(
                            pmk[:, 512 * h:512 * (h + 1)],
                            lhsT=kbd_s[:, b, a, :],
                            rhs=rhs[:, 512 * h:512 * (h + 1)],
                            start=(a == 0), stop=(a == 31))
                nc.vector.tensor_copy(out=meank_b[b][:], in_=pmk[:])

            # (d, n) layout per (chunk, b): (128, chunk, b, n)
            meank_dn = pp.tile([128, 8, BC, N], F32, tag="meank_dn")
            for b in range(BC):
                for c in range(8):
                    ptr = ps_s.tile([128, 32], F32, tag="tr")
                    nc.tensor.transpose(
                        out=ptr[:],
                        in_=meank_b[b][:, 128 * c:128 * (c + 1)],
                        identity=ident[:N, :N])
                    nc.vector.tensor_copy(out=meank_dn[:, c, b, :], in_=ptr[:])

            # ============ prior logits ============
            ppr = ps_s.tile([1, 128], F32, tag="tr")
            for b in range(BC):
                for c in range(8):
                    nc.tensor.matmul(
                        ppr[:, 32 * b:32 * (b + 1)],
                        lhsT=mxy_db[:, 0, c, b:b + 1],
                        rhs=meank_dn[:, c, b, :],
                        start=(c == 0), stop=(c == 7))
            prior_row = mp.tile([1, 128], F32, tag="prow")
            nc.vector.tensor_copy(out=prior_row[:], in_=ppr[:])
            prd = dp.tile([BC, N], F32)
            nc.sync.dma_start(
                out=prd[:].rearrange("b n -> (b n)").rearrange("(o f) -> o f", o=1),
                in_=prior_row[:])
            prior_rows = mp.tile([BC, N], F32, tag="prior_rows")
            nc.sync.dma_start(out=prior_rows[:], in_=prd[:])

            # log_softmax(prior)
            mx = mp.tile([BC, 1], F32, tag="s1")
            nc.vector.reduce_max(out=mx[:], in_=prior_rows[:], axis=AX)
            nmx = mp.tile([BC, 1], F32, tag="s2")
            nc.scalar.mul(nmx[:], mx[:], -1.0)
            ex = mp.tile([BC, N], F32, tag="row4")
            sm = mp.tile([BC, 1], F32, tag="s3")
            nc.scalar.activation(out=ex[:], in_=prior_rows[:], func=AF.Exp,
                                 bias=nmx[:], scale=1.0, accum_out=sm[:])
            lg = mp.tile([BC, 1], F32, tag="s4")
            nc.scalar.activation(out=lg[:], in_=sm[:], func=AF.Ln)
            pr_out = mp.tile([BC, N], F32, tag="row5")
            nc.vector.tensor_scalar(out=pr_out[:], in0=prior_rows[:],
                                    scalar1=mx[:], scalar2=lg[:],
                                    op0=ALU.subtract, op1=ALU.subtract)
            nc.sync.dma_start(out=prior_o[:], in_=pr_out[:])

            # ============ X_cat_Y = [mean_X; mean_Y] @ W1.T + b1 ============
            pxc = ps_a.tile([BC, D], F32, tag="acc")
            for c in range(16):
                w1tile = wp.tile([128, D], F32, tag="w1tile")
                nc.sync.dma_start(out=w1tile[:], in_=w1t[128 * c:128 * (c + 1), :])
                for h in range(2):
                    nc.tensor.matmul(
                        pxc[:, 512 * h:512 * (h + 1)],
                        lhsT=mxy_db[:, c // 8, c % 8, :],
                        rhs=w1tile[:, 512 * h:512 * (h + 1)],
                        start=(c == 0), stop=(c == 15))
            b1b = mp.tile([BC, D], F32, tag="b1b")
            nc.sync.dma_start(out=b1b[:], in_=_bcast_ap(b1r.ap(), BC, D))
            xc_rows = pp.tile([BC, D], F32, tag="xc_rows")
            nc.vector.tensor_add(out=xc_rows[:], in0=pxc[:], in1=b1b[:])
            xc_db = pp.tile([128, 8, BC], F32, tag="xc_db")
            for c in range(8):
                ptr = ps_s.tile([128, 32], F32, tag="tr")
                nc.tensor.transpose(out=ptr[:, :BC],
                                    in_=xc_rows[:, 128 * c:128 * (c + 1)],
                                    identity=ident[:BC, :BC])
                nc.vector.tensor_copy(out=xc_db[:, c, :], in_=ptr[:, :BC])

            # ============ posterior logits ============
            ppo = ps_s.tile([1, 128], F32, tag="tr")
            for b in range(BC):
                for c in range(8):
                    nc.tensor.matmul(
                        ppo[:, 32 * b:32 * (b + 1)],
                        lhsT=xc_db[:, c, b:b + 1],
                        rhs=meank_dn[:, c, b, :],
                        start=(c == 0), stop=(c == 7))
            post_row = mp.tile([1, 128], F32, tag="prow")
            nc.vector.tensor_copy(out=post_row[:], in_=ppo[:])
            pod = dp.tile([BC, N], F32)
            nc.sync.dma_start(
                out=pod[:].rearrange("b n -> (b n)").rearrange("(o f) -> o f", o=1),
                in_=post_row[:])
            post_rows = mp.tile([BC, N], F32, tag="post_rows")
            nc.sync.dma_start(out=post_rows[:], in_=pod[:])

            # posterior = softmax(post_rows)
            mx2 = mp.tile([BC, 1], F32, tag="s1b")
            nc.vector.reduce_max(out=mx2[:], in_=post_rows[:], axis=AX)
            nmx2 = mp.tile([BC, 1], F32, tag="s2b")
            nc.scalar.mul(nmx2[:], mx2[:], -1.0)
            ex2 = mp.tile([BC, N], F32, tag="row6")
            sm2 = mp.tile([BC, 1], F32, tag="s3b")
            nc.scalar.activation(out=ex2[:], in_=post_rows[:], func=AF.Exp,
                                 bias=nmx2[:], scale=1.0, accum_out=sm2[:])
            rc2 = mp.tile([BC, 1], F32, tag="s4b")
            nc.vector.reciprocal(out=rc2[:], in_=sm2[:])
            po_out = mp.tile([BC, N], F32, tag="row7")
            nc.vector.tensor_scalar_mul(po_out[:], ex2[:], rc2[:])
            nc.sync.dma_start(out=post_o[:], in_=po_out[:])

            # K_index = softmax((post_logits + gumbel)/tau)
            gum_s = mp.tile([BC, N], F32, tag="gum_s")
            nc.sync.dma_start(out=gum_s[:], in_=gum[:])
            kg = mp.tile([BC, N], F32, tag="row8")
            nc.vector.tensor_add(out=kg[:], in0=post_rows[:], in1=gum_s[:])
            mx3 = mp.tile([BC, 1], F32, tag="s1c")
            nc.vector.reduce_max(out=mx3[:], in_=kg[:], axis=AX)
            nmx3 = mp.tile([BC, 1], F32, tag="s2c")
            nc.scalar.mul(nmx3[:], mx3[:], -1.0 / TAU)
            ex3 = mp.tile([BC, N], F32, tag="row9")
            sm3 = mp.tile([BC, 1], F32, tag="s3c")
            nc.scalar.activation(out=ex3[:], in_=kg[:], func=AF.Exp,
                                 bias=nmx3[:], scale=1.0 / TAU, accum_out=sm3[:])
            rc3 = mp.tile([BC, 1], F32, tag="s4c")
            nc.vector.reciprocal(out=rc3[:], in_=sm3[:])
            kidx_rows = pp.tile([BC, N], F32, tag="kidx_rows")
            nc.vector.tensor_scalar_mul(kidx_rows[:], ex3[:], rc3[:])
            nc.sync.dma_start(out=kidx_o[:], in_=kidx_rows[:])

            # ============ selected_K: gather argmax slice ============
            selw = mp.tile([BC, 1], F32, tag="selw")
            nc.vector.reduce_max(out=selw[:], in_=kidx_rows[:], axis=AX)
            ind = mp.tile([BC, N], F32, tag="row10")
            nc.vector.tensor_scalar(out=ind[:], in0=kidx_rows[:], scalar1=selw[:],
                                    scalar2=None, op0=ALU.is_equal)
            io32 = mp.tile([BC, N], F32, tag="io32")
            nc.sync.dma_start(out=io32[:], in_=_bcast_ap(iota32.ap(), BC, N))
            nc.vector.tensor_mul(out=ind[:], in0=ind[:], in1=io32[:])
            nsf = mp.tile([BC, 1], F32, tag="nsf")
            nc.vector.reduce_sum(out=nsf[:], in_=ind[:], axis=AX)
            # bounce [n*, w*] to DRAM for per-partition broadcasts
            nsw = dp.tile([BC, 2], F32)
            nc.sync.dma_start(out=nsw[:, 0:1], in_=nsf[:])
            nc.sync.dma_start(out=nsw[:, 1:2], in_=selw[:])
            nsw_ap = nsw[:]
            for b in range(BC):
                nsb = mp.tile([128, 1], F32, tag="nsb")
                nc.sync.dma_start(
                    out=nsb[:],
                    in_=bass.AP(tensor=nsw_ap.tensor, offset=nsw_ap.offset + 2 * b,
                                ap=[[0, 128], [1, 1]]))
                wsb = mp.tile([128, 1], F32, tag="wsb")
                nc.sync.dma_start(
                    out=wsb[:],
                    in_=bass.AP(tensor=nsw_ap.tensor, offset=nsw_ap.offset + 2 * b + 1,
                                ap=[[0, 128], [1, 1]]))
                idxf = mp.tile([128, 1], F32, tag="idxf")
                nc.scalar.mul(idxf[:], nsb[:], float(KS))
                nc.vector.tensor_add(out=idxf[:], in0=idxf[:], in1=iota128_s[:])
                idxi = mp.tile([128, 1], I32, tag="idxi")
                nc.vector.tensor_copy(out=idxi[:], in_=idxf[:])
                selt = selp.tile([128, D], F32, tag="selt")
                nc.gpsimd.indirect_dma_start(
                    out=selt[:], out_offset=None,
                    in_=kh_flat,
                    in_offset=bass.IndirectOffsetOnAxis(ap=idxi[:, :1], axis=0),
                    element_offset=b * D)
                wmt = mp.tile([128, 1], F32, tag="wmt")
                nc.gpsimd.indirect_dma_start(
                    out=wmt[:], out_offset=None,
                    in_=wk_flat,
                    in_offset=bass.IndirectOffsetOnAxis(ap=idxi[:, :1], axis=0),
                    element_offset=b)
                wsel = mp.tile([128, 1], F32, tag="wsel")
                nc.vector.tensor_mul(out=wsel[:], in0=wmt[:], in1=wsb[:])
                nc.scalar.mul(wsel[:], wsel[:], float(KS))
                selo = selp.tile([128, D], F32, tag="selo")
                nc.vector.tensor_scalar_mul(selo[:], selt[:], wsel[:])
                nc.sync.dma_start(out=sel_o[b, :, :], in_=selo[:])

            # ============ vs = sum_n K_index * mean_K ; AllGather ============
            pkt = ps_s.tile([32, BC], F32, tag="tr")
            nc.tensor.transpose(out=pkt[:N, :BC], in_=kidx_rows[:], identity=ident[:BC, :BC])
            kidx_t = mp.tile([N, BC], F32, tag="kidx_t")
            nc.vector.tensor_copy(out=kidx_t[:], in_=pkt[:N, :BC])

            agi = dp.tile([BC, D], F32)
            ago = dp.tile([B, D], F32, addr_space="Shared")
            for b in range(BC):
                pvs = ps_a.tile([1, D], F32, tag="acc")
                for h in range(2):
                    nc.tensor.matmul(
                        pvs[:, 512 * h:512 * (h + 1)],
                        lhsT=kidx_t[:, b:b + 1],
                        rhs=meank_b[b][:, 512 * h:512 * (h + 1)],
                        start=True, stop=True)
                vsr = mp.tile([1, D], F32, tag="vsr")
                nc.vector.tensor_copy(out=vsr[:], in_=pvs[:])
                nc.sync.dma_start(out=agi[b:b + 1, :], in_=vsr[:])
            nc.gpsimd.collective_compute(
                "AllGather", ALU.bypass,
                replica_groups=[list(range(NC))],
                ins=[agi[:].opt()], outs=[ago[:].opt()])
            vs_all = pp.tile([B, D], F32, tag="vs_all")
            nc.sync.dma_start(out=vs_all[:], in_=ago[:])
            vs_db = pp.tile([128, 8, B], F32, tag="vs_db")
            for c in range(8):
                ptr = ps_s.tile([128, 32], F32, tag="tr")
                nc.tensor.transpose(out=ptr[:],
                                    in_=vs_all[:, 128 * c:128 * (c + 1)],
                                    identity=ident[:B, :B])
                nc.vector.tensor_copy(out=vs_db[:, c, :], in_=ptr[:])

            # ============ vocab projection + sharded log_softmax ============
            VH = VC // 2  # 2000
            logits = [pp.tile([B, VH], F32, tag=f"logits{vh}", name=f"logits{vh}")
                      for vh in range(2)]
            parts = []
            for vh in range(2):
                pv = ps_v.tile([B, 4, 512], F32, tag="voc")
                for dc in range(8):
                    w2tile = wp.tile([128, VH], F32, tag="w2tile")
                    nc.sync.dma_start(
                        out=w2tile[:], in_=w2t[128 * dc:128 * (dc + 1),
                                              VH * vh:VH * (vh + 1)])
                    for vt in range(4):
                        nc.tensor.matmul(
                            pv[:, vt, 0:500],
                            lhsT=vs_db[:, dc, :],
                            rhs=w2tile[:, 500 * vt:500 * (vt + 1)],
                            start=(dc == 0), stop=(dc == 7))
                b2b = bp.tile([B, VH], F32, tag="b2b")
                nc.sync.dma_start(out=b2b[:],
                                  in_=_bcast_ap(b2r.ap(), B, VH, offset=VH * vh))
                for vt in range(4):
                    nc.vector.tensor_add(
                        out=logits[vh][:, 500 * vt:500 * (vt + 1)],
                        in0=pv[:, vt, 0:500],
                        in1=b2b[:, 500 * vt:500 * (vt + 1)])
                exv = bp.tile([B, VH], F32, tag="exv")
                part = mp.tile([B, 1], F32, tag=f"part{vh}")
                nc.scalar.activation(out=exv[:], in_=logits[vh][:], func=AF.Exp,
                                     accum_out=part[:])
                parts.append(part)
            locsum = mp.tile([B, 1], F32, tag="locsum")
            nc.vector.tensor_add(out=locsum[:], in0=parts[0][:], in1=parts[1][:])
            ari = dp.tile([B, 1], F32)
            aro = dp.tile([B, 1], F32, addr_space="Shared")
            nc.sync.dma_start(out=ari[:], in_=locsum[:])
            nc.gpsimd.collective_compute(
                "AllReduce", ALU.add,
                replica_groups=[list(range(NC))],
                ins=[ari[:].opt()], outs=[aro[:].opt()])
            gsum = mp.tile([B, 1], F32, tag="gsum")
            nc.sync.dma_start(out=gsum[:], in_=aro[:])
            glog = mp.tile([B, 1], F32, tag="glog")
            nc.scalar.activation(out=glog[:], in_=gsum[:], func=AF.Ln)
            for vh in range(2):
                outv = bp.tile([B, VH], F32, tag="outv")
                nc.vector.tensor_scalar_sub(outv[:], logits[vh][:], glog[:])
                nc.sync.dma_start(out=vocab_o[:, VH * vh:VH * (vh + 1)], in_=outv[:])

    return nc


_NC_CACHE = None


def _get_program():
    global _NC_CACHE
    if _NC_CACHE is None:
        _NC_CACHE = build_program()
    return _NC_CACHE


def kernel(utterence_hidden, response_hidden, knowledge_hidden,
           utterence_pad_mask, response_pad_mask, knowledge_pad_mask,
           gumbel, W1, b1, W2, b2):
    global LAST_RESULTS
    f32 = np.float32
    uh = np.asarray(utterence_hidden, f32)
    rh = np.asarray(response_hidden, f32)
    kh = np.asarray(knowledge_hidden, f32)
    wu = ((~np.asarray(utterence_pad_mask, bool)).astype(f32) / XS)   # (B, XS)
    wr = ((~np.asarray(response_pad_mask, bool)).astype(f32) / YS)    # (B, YS)
    wk = ((~np.asarray(knowledge_pad_mask, bool)).astype(f32) / KS)   # (B, N, KS)
    gm = np.asarray(gumbel, f32)
    w1t = np.ascontiguousarray(np.asarray(W1, f32).T)                 # (2D, D)
    b1r = np.asarray(b1, f32).reshape(1, D)
    W2 = np.asarray(W2, f32)
    b2 = np.asarray(b2, f32)
    iota128 = np.arange(128, dtype=f32).reshape(128, 1)
    iota32 = np.arange(N, dtype=f32).reshape(1, N)

    in_maps = []
    for c in range(NC):
        bs = slice(BC * c, BC * (c + 1))
        bset = range(BC * c, BC * (c + 1))
        # block-diagonal lhsT for mean_X/mean_Y: (128, src, step, m)
        ubd = np.zeros((128, 2, 8, BC), f32)
        for src_i, w in enumerate((wu, wr)):
            for scn in range(2):
                for a in range(4):
                    step = scn * 4 + a
                    s0 = scn * 128 + 32 * a
                    for m, bg in enumerate(bset):
                        ubd[32 * m:32 * (m + 1), src_i, step, m] = w[bg, s0:s0 + 32]
        # block-diagonal lhsT for mean_K: (128, b, arr32, m32);
        # rows [4m:4m+4) of column m hold wk[bg, m, 4a:4a+4]
        kbd = np.zeros((128, BC, 32, 32), f32)
        midx = np.arange(32)
        for bl, bg in enumerate(bset):
            z = np.zeros((32, 4, 32, 32), f32)            # (m, j, a, m')
            z[midx, :, :, midx] = wk[bg].reshape(32, 32, 4).transpose(0, 2, 1)
            kbd[:, bl, :, :] = z.reshape(128, 32, 32)
        vsl = slice(VC * c, VC * (c + 1))
        in_maps.append({
            "u": np.ascontiguousarray(uh[:, bs, :]),
            "r": np.ascontiguousarray(rh[:, bs, :]),
            "kh": np.ascontiguousarray(kh[:, :, bs, :]),
            "ubd": ubd,
            "kbd": kbd,
            "wk_nsb": np.ascontiguousarray(wk[bs].transpose(1, 2, 0)),
            "gum": np.ascontiguousarray(gm[bs]),
            "w1t": w1t,
            "b1r": b1r,
            "w2t": np.ascontiguousarray(W2[vsl, :].T),
            "b2r": b2[vsl].reshape(1, VC),
            "iota128": iota128,
            "iota32": iota32,
        })

    nc = _get_program()
    res = run_bass_kernel_spmd(nc, in_maps, core_ids=list(range(NC)),
                               trace=TRACE, **TRACE_KW)
    LAST_RESULTS = res
    rs = res.results
    prior = np.concatenate([rs[c]["prior_o"] for c in range(NC)], axis=0)
    posterior = np.concatenate([rs[c]["post_o"] for c in range(NC)], axis=0)
    k_index = np.concatenate([rs[c]["kidx_o"] for c in range(NC)], axis=0)
    selected = np.concatenate([rs[c]["sel_o"] for c in range(NC)], axis=0)
    vocab = np.concatenate([rs[c]["vocab_o"] for c in range(NC)], axis=1)
    return (prior, posterior, k_index, selected, vocab)


# revision 12
# speedup vs baseline: 1.2159x; 1.2159x over previous
"""Trainium2 Bass kernel for nn_Knowledge_Manager (moe_routing).

Sharding: data-parallel over batch (32 batches -> 4 per core) for everything
except the 32000-vocab output projection, which is tensor-parallel over the
vocab dim (4000 per core) with a sharded log_softmax (AllGather of the
selected knowledge vector + AllReduce of the per-core sum-of-exp).

Self-contained: builds the Bass program, shards the full inputs, runs on the
8 NeuronCores via run_bass_kernel_spmd, and reassembles full outputs.
"""
import numpy as np
import orjson

import concourse.bass as bass
import concourse.tile as tile
from concourse import mybir
from concourse.bass_utils import run_bass_kernel_spmd
from concourse.masks import make_identity

F32 = mybir.dt.float32
I32 = mybir.dt.int32
AX = mybir.AxisListType.X
AF = mybir.ActivationFunctionType
ALU = mybir.AluOpType

B, N, XS, YS, KS, D, V = 32, 32, 256, 256, 128, 1024, 32000
NC = 8            # cores
BC = B // NC      # batches per core (4)
VC = V // NC      # vocab slice per core (4000)
TAU = 1e-4

# knobs for the dev harness (test.py); the grading path leaves them alone
TRACE = False
TRACE_KW = {}
LAST_RESULTS = None

_ctr = [0]


def _fix_bir_json(data: bytes) -> bytes:
    # This container's walrus encodes at most ONE semaphore wait per
    # instruction; split extra waits onto prefix same-engine NoOps.
    j = orjson.loads(data)
    changed = False
    for f in j.get("functions", []):
        for b in f.get("blocks", []):
            out = []
            for ins in b.get("instructions", []):
                si = ins.get("sync_info")
                waits = si.get("on_wait") if si else None
                if waits and len(waits) > 1:
                    changed = True
                    for w in waits[:-1]:
                        _ctr[0] += 1
                        out.append({
                            "debug": ins.get("debug"),
                            "engine": ins["engine"],
                            "ins": [],
                            "name": f"I-wfx-{_ctr[0]}",
                            "opcode": "NoOp",
                            "outs": [],
                            "text_hint": "waitfix",
                            "sync_info": {"on_update": [], "on_wait": [w]},
                        })
                    si["on_wait"] = [waits[-1]]
                out.append(ins)
            b["instructions"] = out
    return orjson.dumps(j) if changed else data


class _FixedBass(bass.Bass):
    def to_json_bytes(self) -> bytes:
        return _fix_bir_json(super().to_json_bytes())


def _bcast_ap(handle_ap, nparts, nfree, offset=0):
    """(1, nfree) DRAM source broadcast to nparts partitions."""
    return bass.AP(tensor=handle_ap.tensor, offset=offset,
                   ap=[[0, nparts], [1, nfree]])


def build_program():
    nc = _FixedBass()

    # ---- per-core inputs ----
    u = nc.dram_tensor("u", [XS, BC, D], F32, kind="ExternalInput")
    r = nc.dram_tensor("r", [YS, BC, D], F32, kind="ExternalInput")
    kh = nc.dram_tensor("kh", [BC, N, KS, D], F32, kind="ExternalInput")
    ubd = nc.dram_tensor("ubd", [128, 2, 8, BC], F32, kind="ExternalInput")
    kbd = nc.dram_tensor("kbd", [128, BC, 32, 32], F32, kind="ExternalInput")
    wk_nsb = nc.dram_tensor("wk_nsb", [N, KS, BC], F32, kind="ExternalInput")
    gum = nc.dram_tensor("gum", [BC, N], F32, kind="ExternalInput")
    w1t = nc.dram_tensor("w1t", [2 * D, D], F32, kind="ExternalInput")
    b1r = nc.dram_tensor("b1r", [1, D], F32, kind="ExternalInput")
    w2t = nc.dram_tensor("w2t", [D, VC], F32, kind="ExternalInput")
    b2r = nc.dram_tensor("b2r", [1, VC], F32, kind="ExternalInput")
    iota128 = nc.dram_tensor("iota128", [128, 1], F32, kind="ExternalInput")
    iota32 = nc.dram_tensor("iota32", [1, N], F32, kind="ExternalInput")

    # ---- per-core outputs ----
    prior_o = nc.dram_tensor("prior_o", [BC, N], F32, kind="ExternalOutput")
    post_o = nc.dram_tensor("post_o", [BC, N], F32, kind="ExternalOutput")
    kidx_o = nc.dram_tensor("kidx_o", [BC, N], F32, kind="ExternalOutput")
    sel_o = nc.dram_tensor("sel_o", [BC, KS, D], F32, kind="ExternalOutput")
    vocab_o = nc.dram_tensor("vocab_o", [B, VC], F32, kind="ExternalOutput")

    kh_flat = kh.rearrange("b n s d -> (b n s) d")
    wk_flat = wk_nsb.rearrange("n s b -> (n s) b")

    with tile.TileContext(nc) as tc:
        with (
            tc.tile_pool(name="persist", bufs=1) as pp,
            tc.tile_pool(name="stream", bufs=4) as sp,
            tc.tile_pool(name="wstream", bufs=3) as wp,
            tc.tile_pool(name="small", bufs=2) as mp,
            tc.tile_pool(name="selp", bufs=2) as selp,
            tc.tile_pool(name="big1", bufs=1) as bp,
            tc.tile_pool(name="dram", bufs=1, space="DRAM") as dp,
            tc.tile_pool(name="ps_a", bufs=1, space="PSUM") as ps_a,
            tc.tile_pool(name="ps_s", bufs=2, space="PSUM") as ps_s,
            tc.tile_pool(name="ps_v", bufs=1, space="PSUM") as ps_v,
        ):
            ident = pp.tile([128, 128], F32, tag="ident")
            make_identity(nc, ident)

            # persistent small inputs
            ubd_s = pp.tile([128, 2, 8, BC], F32, tag="ubd_s")
            nc.sync.dma_start(out=ubd_s[:], in_=ubd[:])
            kbd_s = pp.tile([128, BC, 32, 32], F32, tag="kbd_s")
            nc.sync.dma_start(out=kbd_s[:], in_=kbd[:])
            iota128_s = pp.tile([128, 1], F32, tag="iota128_s")
            nc.sync.dma_start(out=iota128_s[:], in_=iota128[:])

            # ============ mean_X / mean_Y ============
            # masked mean over sequence via block-diagonal packed matvecs:
            # psum rows (BC, D) accumulate 8 steps (2 s-chunks x 4 arrangements)
            mxy_rows = []
            for src_i, src in enumerate((u, r)):
                pxy = ps_a.tile([BC, D], F32, tag="acc")
                for sc in range(2):
                    for a in range(4):
                        s0 = sc * 128 + 32 * a
                        rhs = sp.tile([128, D], F32, tag="rhs")
                        nc.sync.dma_start(
                            out=rhs[:],
                            in_=src[s0:s0 + 32, :, :].rearrange("s b d -> b s d"))
                        step = sc * 4 + a
                        for h in range(2):
                            nc.tensor.matmul(
                                pxy[:, 512 * h:512 * (h + 1)],
                                lhsT=ubd_s[:, src_i, step, :],
                                rhs=rhs[:, 512 * h:512 * (h + 1)],
                                start=(step == 0), stop=(step == 7))
                rows = pp.tile([BC, D], F32, tag=f"mxy{src_i}")
                nc.vector.tensor_copy(out=rows[:], in_=pxy[:])
                mxy_rows.append(rows)

            # transpose to (d, b) column layout: (128, src, chunk, b)
            mxy_db = pp.tile([128, 2, 8, BC], F32, tag="mxy_db")
            for src_i in range(2):
                for c in range(8):
                    ptr = ps_s.tile([128, 32], F32, tag="tr")
                    nc.tensor.transpose(
                        out=ptr[:, :BC], in_=mxy_rows[src_i][:, 128 * c:128 * (c + 1)],
                        identity=ident[:BC, :BC])
                    nc.vector.tensor_copy(out=mxy_db[:, src_i, c, :], in_=ptr[:, :BC])

            # ============ mean_K ============
            # per-batch (32 n, D) tiles; 32 accumulating block-diagonal matmuls
            meank_b = [pp.tile([N, D], F32, tag=f"meank_b{b}", name=f"meank_b{b}")
                       for b in range(BC)]
            for b in range(BC):
                pmk = ps_a.tile([N, D], F32, tag="acc")
                for a in range(32):  # 4-row s-arrangements
                    rhs = sp.tile([128, D], F32, tag="rhs")
                    nc.sync.dma_start(
                        out=rhs[:],
                        in_=kh[b, :, 4 * a:4 * (a + 1), :])
                    for h in range(2):
                        nc.tensor.matmul(
                            pmk[:, 512 * h:512 * (h + 1)],
                            lhsT=kbd_s[:, b, a, :],
                            rhs=rhs[:, 512 * h:512 * (h + 1)],
                            start=(a == 0), stop=(a == 31))
                nc.vector.tensor_copy(out=meank_b[b][:], in_=pmk[:])

            # (d, n) layout per (chunk, b): (128, chunk, b, n)
            meank_dn = pp.tile([128, 8, BC, N], F32, tag="meank_dn")
            for b in range(BC):
                for c in range(8):
                    ptr = ps_s.tile([128, 32], F32, tag="tr")
                    nc.tensor.transpose(
                        out=ptr[:],
                        in_=meank_b[b][:, 128 * c:128 * (c + 1)],
                        identity=ident[:N, :N])
                    nc.vector.tensor_copy(out=meank_dn[:, c, b, :], in_=ptr[:])

            # ============ prior logits ============
            ppr = ps_s.tile([1, 128], F32, tag="tr")
            for b in range(BC):
                for c in range(8):
                    nc.tensor.matmul(
                        ppr[:, 32 * b:32 * (b + 1)],
                        lhsT=mxy_db[:, 0, c, b:b + 1],
                        rhs=meank_dn[:, c, b, :],
                        start=(c == 0), stop=(c == 7))
            prior_row = mp.tile([1, 128], F32, tag="prow")
            nc.vector.tensor_copy(out=prior_row[:], in_=ppr[:])
            prd = dp.tile([BC, N], F32)
            nc.sync.dma_start(
                out=prd[:].rearrange("b n -> (b n)").rearrange("(o f) -> o f", o=1),
                in_=prior_row[:])
            prior_rows = mp.tile([BC, N], F32, tag="prior_rows")
            nc.sync.dma_start(out=prior_rows[:], in_=prd[:])

            # log_softmax(prior)
            mx = mp.tile([BC, 1], F32, tag="s1")
            nc.vector.reduce_max(out=mx[:], in_=prior_rows[:], axis=AX)
            nmx = mp.tile([BC, 1], F32, tag="s2")
            nc.scalar.mul(nmx[:], mx[:], -1.0)
            ex = mp.tile([BC, N], F32, tag="row4")
            sm = mp.tile([BC, 1], F32, tag="s3")
            nc.scalar.activation(out=ex[:], in_=prior_rows[:], func=AF.Exp,
                                 bias=nmx[:], scale=1.0, accum_out=sm[:])
            lg = mp.tile([BC, 1], F32, tag="s4")
            nc.scalar.activation(out=lg[:], in_=sm[:], func=AF.Ln)
            pr_out = mp.tile([BC, N], F32, tag="row5")
            nc.vector.tensor_scalar(out=pr_out[:], in0=prior_rows[:],
                                    scalar1=mx[:], scalar2=lg[:],
                                    op0=ALU.subtract, op1=ALU.subtract)
            nc.sync.dma_start(out=prior_o[:], in_=pr_out[:])

            # ============ X_cat_Y = [mean_X; mean_Y] @ W1.T + b1 ============
            pxc = ps_a.tile([BC, D], F32, tag="acc")
            for c in range(16):
                w1tile = wp.tile([128, D], F32, tag="w1tile")
                nc.sync.dma_start(out=w1tile[:], in_=w1t[128 * c:128 * (c + 1), :])
                for h in range(2):
                    nc.tensor.matmul(
                        pxc[:, 512 * h:512 * (h + 1)],
                        lhsT=mxy_db[:, c // 8, c % 8, :],
                        rhs=w1tile[:, 512 * h:512 * (h + 1)],
                        start=(c == 0), stop=(c == 15))
            b1b = mp.tile([BC, D], F32, tag="b1b")
            nc.sync.dma_start(out=b1b[:], in_=_bcast_ap(b1r.ap(), BC, D))
            xc_rows = pp.tile([BC, D], F32, tag="xc_rows")
            nc.vector.tensor_add(out=xc_rows[:], in0=pxc[:], in1=b1b[:])
            xc_db = pp.tile([128, 8, BC], F32, tag="xc_db")
            for c in range(8):
                ptr = ps_s.tile([128, 32], F32, tag="tr")
                nc.tensor.transpose(out=ptr[:, :BC],
                                    in_=xc_rows[:, 128 * c:128 * (c + 1)],
                                    identity=ident[:BC, :BC])
                nc.vector.tensor_copy(out=xc_db[:, c, :], in_=ptr[:, :BC])

            # ============ posterior logits ============
            ppo = ps_s.tile([1, 128], F32, tag="tr")
            for b in range(BC):
                for c in range(8):
                    nc.tensor.matmul(
                        ppo[:, 32 * b:32 * (b + 1)],
                        lhsT=xc_db[:, c, b:b + 1],
                        rhs=meank_dn[:, c, b, :],
                        start=(c == 0), stop=(c == 7))
            post_row = mp.tile([1, 128], F32, tag="prow")
            nc.vector.tensor_copy(out=post_row[:], in_=ppo[:])
            pod = dp.tile([BC, N], F32)
            nc.sync.dma_start(
                out=pod[:].rearrange("b n -> (b n)").rearrange("(o f) -> o f", o=1),
                in_=post_row[:])
            post_rows = mp.tile([BC, N], F32, tag="post_rows")
            nc.sync.dma_start(out=post_rows[:], in_=pod[:])

            # posterior = softmax(post_rows)
            mx2 = mp.tile([BC, 1], F32, tag="s1b")
            nc.vector.reduce_max(out=mx2[:], in_=post_rows[:], axis=AX)
            nmx2 = mp.tile([BC, 1], F32, tag="s2b")
            nc.scalar.mul(nmx2[:], mx2[:], -1.0)
            ex2 = mp.tile([BC, N], F32, tag="row6")
            sm2 = mp.tile([BC, 1], F32, tag="s3b")
            nc.scalar.activation(out=ex2[:], in_=post_rows[:], func=AF.Exp,
                                 bias=nmx2[:], scale=1.0, accum_out=sm2[:])
            rc2 = mp.tile([BC, 1], F32, tag="s4b")
            nc.vector.reciprocal(out=rc2[:], in_=sm2[:])
            po_out = mp.tile([BC, N], F32, tag="row7")
            nc.vector.tensor_scalar_mul(po_out[:], ex2[:], rc2[:])
            nc.sync.dma_start(out=post_o[:], in_=po_out[:])

            # K_index = softmax((post_logits + gumbel)/tau)
            gum_s = mp.tile([BC, N], F32, tag="gum_s")
            nc.sync.dma_start(out=gum_s[:], in_=gum[:])
            kg = mp.tile([BC, N], F32, tag="row8")
            nc.vector.tensor_add(out=kg[:], in0=post_rows[:], in1=gum_s[:])
            mx3 = mp.tile([BC, 1], F32, tag="s1c")
            nc.vector.reduce_max(out=mx3[:], in_=kg[:], axis=AX)
            nmx3 = mp.tile([BC, 1], F32, tag="s2c")
            nc.scalar.mul(nmx3[:], mx3[:], -1.0 / TAU)
            ex3 = mp.tile([BC, N], F32, tag="row9")
            sm3 = mp.tile([BC, 1], F32, tag="s3c")
            nc.scalar.activation(out=ex3[:], in_=kg[:], func=AF.Exp,
                                 bias=nmx3[:], scale=1.0 / TAU, accum_out=sm3[:])
            rc3 = mp.tile([BC, 1], F32, tag="s4c")
            nc.vector.reciprocal(out=rc3[:], in_=sm3[:])
            kidx_rows = pp.tile([BC, N], F32, tag="kidx_rows")
            nc.vector.tensor_scalar_mul(kidx_rows[:], ex3[:], rc3[:])
            nc.sync.dma_start(out=kidx_o[:], in_=kidx_rows[:])

            # ============ selected_K: gather argmax slice ============
            selw = mp.tile([BC, 1], F32, tag="selw")
            nc.vector.reduce_max(out=selw[:], in_=kidx_rows[:], axis=AX)
            ind = mp.tile([BC, N], F32, tag="row10")
            nc.vector.tensor_scalar(out=ind[:], in0=kidx_rows[:], scalar1=selw[:],
                                    scalar2=None, op0=ALU.is_equal)
            io32 = mp.tile([BC, N], F32, tag="io32")
            nc.sync.dma_start(out=io32[:], in_=_bcast_ap(iota32.ap(), BC, N))
            nc.vector.tensor_mul(out=ind[:], in0=ind[:], in1=io32[:])
            nsf = mp.tile([BC, 1], F32, tag="nsf")
            nc.vector.reduce_sum(out=nsf[:], in_=ind[:], axis=AX)
            # bounce [n*, w*] to DRAM for per-partition broadcasts
            nsw = dp.tile([BC, 2], F32)
            nc.sync.dma_start(out=nsw[:, 0:1], in_=nsf[:])
            nc.sync.dma_start(out=nsw[:, 1:2], in_=selw[:])
            nsw_ap = nsw[:]
            for b in range(BC):
                nsb = mp.tile([128, 1], F32, tag="nsb")
                nc.sync.dma_start(
                    out=nsb[:],
                    in_=bass.AP(tensor=nsw_ap.tensor, offset=nsw_ap.offset + 2 * b,
                                ap=[[0, 128], [1, 1]]))
                wsb = mp.tile([128, 1], F32, tag="wsb")
                nc.sync.dma_start(
                    out=wsb[:],
                    in_=bass.AP(tensor=nsw_ap.tensor, offset=nsw_ap.offset + 2 * b + 1,
                                ap=[[0, 128], [1, 1]]))
                idxf = mp.tile([128, 1], F32, tag="idxf")
                nc.scalar.mul(idxf[:], nsb[:], float(KS))
                nc.vector.tensor_add(out=idxf[:], in0=idxf[:], in1=iota128_s[:])
                idxw = mp.tile([128, 1], I32, tag="idxw")
                nc.vector.tensor_copy(out=idxw[:], in_=idxf[:])
                nc.vector.tensor_scalar_add(idxf[:], idxf[:], float(b * N * KS))
                idxi = mp.tile([128, 1], I32, tag="idxi")
                nc.vector.tensor_copy(out=idxi[:], in_=idxf[:])
                selt = selp.tile([128, D], F32, tag="selt")
                nc.gpsimd.indirect_dma_start(
                    out=selt[:], out_offset=None,
                    in_=kh_flat,
                    in_offset=bass.IndirectOffsetOnAxis(ap=idxi[:, :1], axis=0))
                wmt = mp.tile([128, 1], F32, tag="wmt")
                nc.gpsimd.indirect_dma_start(
                    out=wmt[:], out_offset=None,
                    in_=wk_flat,
                    in_offset=bass.IndirectOffsetOnAxis(ap=idxw[:, :1], axis=0),
                    element_offset=b)
                wsel = mp.tile([128, 1], F32, tag="wsel")
                nc.vector.tensor_mul(out=wsel[:], in0=wmt[:], in1=wsb[:])
                nc.scalar.mul(wsel[:], wsel[:], float(KS))
                selo = selp.tile([128, D], F32, tag="selo")
                nc.vector.tensor_scalar_mul(selo[:], selt[:], wsel[:])
                nc.sync.dma_start(out=sel_o[b, :, :], in_=selo[:])

            # ============ vs = sum_n K_index * mean_K ; AllGather ============
            pkt = ps_s.tile([32, BC], F32, tag="tr")
            nc.tensor.transpose(out=pkt[:N, :BC], in_=kidx_rows[:], identity=ident[:BC, :BC])
            kidx_t = mp.tile([N, BC], F32, tag="kidx_t")
            nc.vector.tensor_copy(out=kidx_t[:], in_=pkt[:N, :BC])

            agi = dp.tile([BC, D], F32)
            ago = dp.tile([B, D], F32, addr_space="Shared")
            for b in range(BC):
                pvs = ps_a.tile([1, D], F32, tag="acc")
                for h in range(2):
                    nc.tensor.matmul(
                        pvs[:, 512 * h:512 * (h + 1)],
                        lhsT=kidx_t[:, b:b + 1],
                        rhs=meank_b[b][:, 512 * h:512 * (h + 1)],
                        start=True, stop=True)
                vsr = mp.tile([1, D], F32, tag="vsr")
                nc.vector.tensor_copy(out=vsr[:], in_=pvs[:])
                nc.sync.dma_start(out=agi[b:b + 1, :], in_=vsr[:])
            nc.gpsimd.collective_compute(
                "AllGather", ALU.bypass,
                replica_groups=[list(range(NC))],
                ins=[agi[:].opt()], outs=[ago[:].opt()])
            vs_all = pp.tile([B, D], F32, tag="vs_all")
            nc.sync.dma_start(out=vs_all[:], in_=ago[:])
            vs_db = pp.tile([128, 8, B], F32, tag="vs_db")
            for c in range(8):
                ptr = ps_s.tile([128, 32], F32, tag="tr")
                nc.tensor.transpose(out=ptr[:],
                                    in_=vs_all[:, 128 * c:128 * (c + 1)],
                                    identity=ident[:B, :B])
                nc.vector.tensor_copy(out=vs_db[:, c, :], in_=ptr[:])

            # ============ vocab projection + sharded log_softmax ============
            VH = VC // 2  # 2000
            logits = [pp.tile([B, VH], F32, tag=f"logits{vh}", name=f"logits{vh}")
                      for vh in range(2)]
            parts = []
            for vh in range(2):
                pv = ps_v.tile([B, 4, 512], F32, tag="voc")
                for dc in range(8):
                    w2tile = wp.tile([128, VH], F32, tag="w2tile")
                    nc.sync.dma_start(
                        out=w2tile[:], in_=w2t[128 * dc:128 * (dc + 1),
                                              VH * vh:VH * (vh + 1)])
                    for vt in range(4):
                        nc.tensor.matmul(
                            pv[:, vt, 0:500],
                            lhsT=vs_db[:, dc, :],
                            rhs=w2tile[:, 500 * vt:500 * (vt + 1)],
                            start=(dc == 0), stop=(dc == 7))
                b2b = bp.tile([B, VH], F32, tag="b2b")
                nc.sync.dma_start(out=b2b[:],
                                  in_=_bcast_ap(b2r.ap(), B, VH, offset=VH * vh))
                for vt in range(4):
                    nc.vector.tensor_add(
                        out=logits[vh][:, 500 * vt:500 * (vt + 1)],
                        in0=pv[:, vt, 0:500],
                        in1=b2b[:, 500 * vt:500 * (vt + 1)])
                exv = bp.tile([B, VH], F32, tag="exv")
                part = mp.tile([B, 1], F32, tag=f"part{vh}")
                nc.scalar.activation(out=exv[:], in_=logits[vh][:], func=AF.Exp,
                                     accum_out=part[:])
                parts.append(part)
            locsum = mp.tile([B, 1], F32, tag="locsum")
            nc.vector.tensor_add(out=locsum[:], in0=parts[0][:], in1=parts[1][:])
            ari = dp.tile([B, 1], F32)
            aro = dp.tile([B, 1], F32, addr_space="Shared")
            nc.sync.dma_start(out=ari[:], in_=locsum[:])
            nc.gpsimd.collective_compute(
                "AllReduce", ALU.add,
                replica_groups=[list(range(NC))],
                ins=[ari[:].opt()], outs=[aro[:].opt()])
            gsum = mp.tile([B, 1], F32, tag="gsum")
            nc.sync.dma_start(out=gsum[:], in_=aro[:])
            glog = mp.tile([B, 1], F32, tag="glog")
            nc.scalar.activation(out=glog[:], in_=gsum[:], func=AF.Ln)
            for vh in range(2):
                outv = bp.tile([B, VH], F32, tag="outv")
                nc.vector.tensor_scalar_sub(outv[:], logits[vh][:], glog[:])
                nc.sync.dma_start(out=vocab_o[:, VH * vh:VH * (vh + 1)], in_=outv[:])

    return nc


_NC_CACHE = None


def _get_program():
    global _NC_CACHE
    if _NC_CACHE is None:
        _NC_CACHE = build_program()
    return _NC_CACHE


def kernel(utterence_hidden, response_hidden, knowledge_hidden,
           utterence_pad_mask, response_pad_mask, knowledge_pad_mask,
           gumbel, W1, b1, W2, b2):
    global LAST_RESULTS
    f32 = np.float32
    uh = np.asarray(utterence_hidden, f32)
    rh = np.asarray(response_hidden, f32)
    kh = np.asarray(knowledge_hidden, f32)
    wu = ((~np.asarray(utterence_pad_mask, bool)).astype(f32) / XS)   # (B, XS)
    wr = ((~np.asarray(response_pad_mask, bool)).astype(f32) / YS)    # (B, YS)
    wk = ((~np.asarray(knowledge_pad_mask, bool)).astype(f32) / KS)   # (B, N, KS)
    gm = np.asarray(gumbel, f32)
    w1t = np.ascontiguousarray(np.asarray(W1, f32).T)                 # (2D, D)
    b1r = np.asarray(b1, f32).reshape(1, D)
    W2 = np.asarray(W2, f32)
    b2 = np.asarray(b2, f32)
    iota128 = np.arange(128, dtype=f32).reshape(128, 1)
    iota32 = np.arange(N, dtype=f32).reshape(1, N)

    in_maps = []
    for c in range(NC):
        bs = slice(BC * c, BC * (c + 1))
        bset = range(BC * c, BC * (c + 1))
        # block-diagonal lhsT for mean_X/mean_Y: (128, src, step, m)
        ubd = np.zeros((128, 2, 8, BC), f32)
        for src_i, w in enumerate((wu, wr)):
            for scn in range(2):
                for a in range(4):
                    step = scn * 4 + a
                    s0 = scn * 128 + 32 * a
                    for m, bg in enumerate(bset):
                        ubd[32 * m:32 * (m + 1), src_i, step, m] = w[bg, s0:s0 + 32]
        # block-diagonal lhsT for mean_K: (128, b, arr32, m32);
        # rows [4m:4m+4) of column m hold wk[bg, m, 4a:4a+4]
        kbd = np.zeros((128, BC, 32, 32), f32)
        midx = np.arange(32)
        for bl, bg in enumerate(bset):
            z = np.zeros((32, 4, 32, 32), f32)            # (m, j, a, m')
            z[midx, :, :, midx] = wk[bg].reshape(32, 32, 4).transpose(0, 2, 1)
            kbd[:, bl, :, :] = z.reshape(128, 32, 32)
        vsl = slice(VC * c, VC * (c + 1))
        in_maps.append({
            "u": np.ascontiguousarray(uh[:, bs, :]),
            "r": np.ascontiguousarray(rh[:, bs, :]),
            "kh": np.ascontiguousarray(kh[:, :, bs, :].transpose(2, 0, 1, 3)),
            "ubd": ubd,
            "kbd": kbd,
            "wk_nsb": np.ascontiguousarray(wk[bs].transpose(1, 2, 0)),
            "gum": np.ascontiguousarray(gm[bs]),
            "w1t": w1t,
            "b1r": b1r,
            "w2t": np.ascontiguousarray(W2[vsl, :].T),
            "b2r": b2[vsl].reshape(1, VC),
            "iota128": iota128,
            "iota32": iota32,
        })

    nc = _get_program()
    res = run_bass_kernel_spmd(nc, in_maps, core_ids=list(range(NC)),
                               trace=TRACE, **TRACE_KW)
    LAST_RESULTS = res
    rs = res.results
    prior = np.concatenate([rs[c]["prior_o"] for c in range(NC)], axis=0)
    posterior = np.concatenate([rs[c]["post_o"] for c in range(NC)], axis=0)
    k_index = np.concatenate([rs[c]["kidx_o"] for c in range(NC)], axis=0)
    selected = np.concatenate([rs[c]["sel_o"] for c in range(NC)], axis=0)
    vocab = np.concatenate([rs[c]["vocab_o"] for c in range(NC)], axis=1)
    return (prior, posterior, k_index, selected, vocab)


# revision 13
# speedup vs baseline: 1.3258x; 1.0904x over previous
"""Trainium2 Bass kernel for nn_Knowledge_Manager (moe_routing).

Sharding: data-parallel over batch (32 batches -> 4 per core) for everything
except the 32000-vocab output projection, which is tensor-parallel over the
vocab dim (4000 per core) with a sharded log_softmax (AllGather of the
selected knowledge vector + AllReduce of the per-core sum-of-exp).

Self-contained: builds the Bass program, shards the full inputs, runs on the
8 NeuronCores via run_bass_kernel_spmd, and reassembles full outputs.
"""
import numpy as np
import orjson

import concourse.bass as bass
import concourse.tile as tile
from concourse import mybir
from concourse.bass_utils import run_bass_kernel_spmd
from concourse.masks import make_identity

F32 = mybir.dt.float32
I32 = mybir.dt.int32
AX = mybir.AxisListType.X
AF = mybir.ActivationFunctionType
ALU = mybir.AluOpType

B, N, XS, YS, KS, D, V = 32, 32, 256, 256, 128, 1024, 32000
NC = 8            # cores
BC = B // NC      # batches per core (4)
VC = V // NC      # vocab slice per core (4000)
TAU = 1e-4

# knobs for the dev harness (test.py); the grading path leaves them alone
TRACE = False
TRACE_KW = {}
LAST_RESULTS = None

_ctr = [0]


def _fix_bir_json(data: bytes) -> bytes:
    # This container's walrus encodes at most ONE semaphore wait per
    # instruction; split extra waits onto prefix same-engine NoOps.
    j = orjson.loads(data)
    changed = False
    for f in j.get("functions", []):
        for b in f.get("blocks", []):
            out = []
            for ins in b.get("instructions", []):
                si = ins.get("sync_info")
                waits = si.get("on_wait") if si else None
                if waits and len(waits) > 1:
                    changed = True
                    for w in waits[:-1]:
                        _ctr[0] += 1
                        out.append({
                            "debug": ins.get("debug"),
                            "engine": ins["engine"],
                            "ins": [],
                            "name": f"I-wfx-{_ctr[0]}",
                            "opcode": "NoOp",
                            "outs": [],
                            "text_hint": "waitfix",
                            "sync_info": {"on_update": [], "on_wait": [w]},
                        })
                    si["on_wait"] = [waits[-1]]
                out.append(ins)
            b["instructions"] = out
    return orjson.dumps(j) if changed else data


class _FixedBass(bass.Bass):
    def to_json_bytes(self) -> bytes:
        return _fix_bir_json(super().to_json_bytes())


def _bcast_ap(handle_ap, nparts, nfree, offset=0):
    """(1, nfree) DRAM source broadcast to nparts partitions."""
    return bass.AP(tensor=handle_ap.tensor, offset=offset,
                   ap=[[0, nparts], [1, nfree]])


def build_program():
    nc = _FixedBass()

    # ---- per-core inputs ----
    u = nc.dram_tensor("u", [XS, BC, D], F32, kind="ExternalInput")
    r = nc.dram_tensor("r", [YS, BC, D], F32, kind="ExternalInput")
    kh = nc.dram_tensor("kh", [BC, 32, N, 4, D], F32, kind="ExternalInput")
    ubd = nc.dram_tensor("ubd", [128, 2, 8, BC], F32, kind="ExternalInput")
    kbd = nc.dram_tensor("kbd", [128, BC, 32, 32], F32, kind="ExternalInput")
    wk_nsb = nc.dram_tensor("wk_nsb", [N, KS, BC], F32, kind="ExternalInput")
    gum = nc.dram_tensor("gum", [BC, N], F32, kind="ExternalInput")
    w1t = nc.dram_tensor("w1t", [2 * D, D], F32, kind="ExternalInput")
    b1r = nc.dram_tensor("b1r", [1, D], F32, kind="ExternalInput")
    w2t = nc.dram_tensor("w2t", [D, VC], F32, kind="ExternalInput")
    b2r = nc.dram_tensor("b2r", [1, VC], F32, kind="ExternalInput")
    iota128 = nc.dram_tensor("iota128", [128, 1], F32, kind="ExternalInput")
    iotakh = nc.dram_tensor("iotakh", [128, 1], F32, kind="ExternalInput")
    iota32 = nc.dram_tensor("iota32", [1, N], F32, kind="ExternalInput")

    # ---- per-core outputs ----
    prior_o = nc.dram_tensor("prior_o", [BC, N], F32, kind="ExternalOutput")
    post_o = nc.dram_tensor("post_o", [BC, N], F32, kind="ExternalOutput")
    kidx_o = nc.dram_tensor("kidx_o", [BC, N], F32, kind="ExternalOutput")
    sel_o = nc.dram_tensor("sel_o", [BC, KS, D], F32, kind="ExternalOutput")
    vocab_o = nc.dram_tensor("vocab_o", [B, VC], F32, kind="ExternalOutput")

    kh_flat = kh.rearrange("b a n j d -> (b a n j) d")
    wk_flat = wk_nsb.rearrange("n s b -> (n s) b")

    with tile.TileContext(nc) as tc:
        with (
            tc.tile_pool(name="persist", bufs=1) as pp,
            tc.tile_pool(name="stream", bufs=8) as sp,
            tc.tile_pool(name="wstream", bufs=3) as wp,
            tc.tile_pool(name="small", bufs=2) as mp,
            tc.tile_pool(name="selp", bufs=2) as selp,
            tc.tile_pool(name="big1", bufs=1) as bp,
            tc.tile_pool(name="dram", bufs=1, space="DRAM") as dp,
            tc.tile_pool(name="ps_a", bufs=1, space="PSUM") as ps_a,
            tc.tile_pool(name="ps_s", bufs=2, space="PSUM") as ps_s,
            tc.tile_pool(name="ps_v", bufs=1, space="PSUM") as ps_v,
        ):
            ident = pp.tile([128, 128], F32, tag="ident")
            make_identity(nc, ident)

            # persistent small inputs
            ubd_s = pp.tile([128, 2, 8, BC], F32, tag="ubd_s")
            nc.sync.dma_start(out=ubd_s[:], in_=ubd[:])
            kbd_s = pp.tile([128, BC, 32, 32], F32, tag="kbd_s")
            nc.sync.dma_start(out=kbd_s[:], in_=kbd[:])
            iota128_s = pp.tile([128, 1], F32, tag="iota128_s")
            nc.sync.dma_start(out=iota128_s[:], in_=iota128[:])
            iotakh_s = pp.tile([128, 1], F32, tag="iotakh_s")
            nc.sync.dma_start(out=iotakh_s[:], in_=iotakh[:])

            # ============ mean_X / mean_Y ============
            # masked mean over sequence via block-diagonal packed matvecs:
            # psum rows (BC, D) accumulate 8 steps (2 s-chunks x 4 arrangements)
            mxy_rows = []
            for src_i, src in enumerate((u, r)):
                pxy = ps_a.tile([BC, D], F32, tag="acc")
                for sc in range(2):
                    for a in range(4):
                        s0 = sc * 128 + 32 * a
                        rhs = sp.tile([128, D], F32, tag="rhs")
                        nc.sync.dma_start(
                            out=rhs[:],
                            in_=src[s0:s0 + 32, :, :].rearrange("s b d -> b s d"))
                        step = sc * 4 + a
                        for h in range(2):
                            nc.tensor.matmul(
                                pxy[:, 512 * h:512 * (h + 1)],
                                lhsT=ubd_s[:, src_i, step, :],
                                rhs=rhs[:, 512 * h:512 * (h + 1)],
                                start=(step == 0), stop=(step == 7))
                rows = pp.tile([BC, D], F32, tag=f"mxy{src_i}")
                nc.vector.tensor_copy(out=rows[:], in_=pxy[:])
                mxy_rows.append(rows)

            # transpose to (d, b) column layout: (128, src, chunk, b)
            mxy_db = pp.tile([128, 2, 8, BC], F32, tag="mxy_db")
            for src_i in range(2):
                for c in range(8):
                    ptr = ps_s.tile([128, 32], F32, tag="tr")
                    nc.tensor.transpose(
                        out=ptr[:, :BC], in_=mxy_rows[src_i][:, 128 * c:128 * (c + 1)],
                        identity=ident[:BC, :BC])
                    nc.vector.tensor_copy(out=mxy_db[:, src_i, c, :], in_=ptr[:, :BC])

            # ============ mean_K ============
            # per-batch (32 n, D) tiles; 32 accumulating block-diagonal matmuls
            meank_b = [pp.tile([N, D], F32, tag=f"meank_b{b}", name=f"meank_b{b}")
                       for b in range(BC)]
            for b in range(BC):
                pmk = ps_a.tile([N, D], F32, tag="acc")
                for a in range(32):  # 4-row s-arrangements
                    rhs = sp.tile([128, D], F32, tag="rhs")
                    nc.sync.dma_start(
                        out=rhs[:],
                        in_=kh[b, a, :, :, :])
                    for h in range(2):
                        nc.tensor.matmul(
                            pmk[:, 512 * h:512 * (h + 1)],
                            lhsT=kbd_s[:, b, a, :],
                            rhs=rhs[:, 512 * h:512 * (h + 1)],
                            start=(a == 0), stop=(a == 31))
                nc.vector.tensor_copy(out=meank_b[b][:], in_=pmk[:])

            # (d, n) layout per (chunk, b): (128, chunk, b, n)
            meank_dn = pp.tile([128, 8, BC, N], F32, tag="meank_dn")
            for b in range(BC):
                for c in range(8):
                    ptr = ps_s.tile([128, 32], F32, tag="tr")
                    nc.tensor.transpose(
                        out=ptr[:],
                        in_=meank_b[b][:, 128 * c:128 * (c + 1)],
                        identity=ident[:N, :N])
                    nc.vector.tensor_copy(out=meank_dn[:, c, b, :], in_=ptr[:])

            # ============ prior logits ============
            ppr = ps_s.tile([1, 128], F32, tag="tr")
            for b in range(BC):
                for c in range(8):
                    nc.tensor.matmul(
                        ppr[:, 32 * b:32 * (b + 1)],
                        lhsT=mxy_db[:, 0, c, b:b + 1],
                        rhs=meank_dn[:, c, b, :],
                        start=(c == 0), stop=(c == 7))
            prior_row = mp.tile([1, 128], F32, tag="prow")
            nc.vector.tensor_copy(out=prior_row[:], in_=ppr[:])
            prd = dp.tile([BC, N], F32)
            nc.sync.dma_start(
                out=prd[:].rearrange("b n -> (b n)").rearrange("(o f) -> o f", o=1),
                in_=prior_row[:])
            prior_rows = mp.tile([BC, N], F32, tag="prior_rows")
            nc.sync.dma_start(out=prior_rows[:], in_=prd[:])

            # log_softmax(prior)
            mx = mp.tile([BC, 1], F32, tag="s1")
            nc.vector.reduce_max(out=mx[:], in_=prior_rows[:], axis=AX)
            nmx = mp.tile([BC, 1], F32, tag="s2")
            nc.scalar.mul(nmx[:], mx[:], -1.0)
            ex = mp.tile([BC, N], F32, tag="row4")
            sm = mp.tile([BC, 1], F32, tag="s3")
            nc.scalar.activation(out=ex[:], in_=prior_rows[:], func=AF.Exp,
                                 bias=nmx[:], scale=1.0, accum_out=sm[:])
            lg = mp.tile([BC, 1], F32, tag="s4")
            nc.scalar.activation(out=lg[:], in_=sm[:], func=AF.Ln)
            pr_out = mp.tile([BC, N], F32, tag="row5")
            nc.vector.tensor_scalar(out=pr_out[:], in0=prior_rows[:],
                                    scalar1=mx[:], scalar2=lg[:],
                                    op0=ALU.subtract, op1=ALU.subtract)
            nc.sync.dma_start(out=prior_o[:], in_=pr_out[:])

            # ============ X_cat_Y = [mean_X; mean_Y] @ W1.T + b1 ============
            pxc = ps_a.tile([BC, D], F32, tag="acc")
            for c in range(16):
                w1tile = wp.tile([128, D], F32, tag="w1tile")
                nc.sync.dma_start(out=w1tile[:], in_=w1t[128 * c:128 * (c + 1), :])
                for h in range(2):
                    nc.tensor.matmul(
                        pxc[:, 512 * h:512 * (h + 1)],
                        lhsT=mxy_db[:, c // 8, c % 8, :],
                        rhs=w1tile[:, 512 * h:512 * (h + 1)],
                        start=(c == 0), stop=(c == 15))
            b1b = mp.tile([BC, D], F32, tag="b1b")
            nc.sync.dma_start(out=b1b[:], in_=_bcast_ap(b1r.ap(), BC, D))
            xc_rows = pp.tile([BC, D], F32, tag="xc_rows")
            nc.vector.tensor_add(out=xc_rows[:], in0=pxc[:], in1=b1b[:])
            xc_db = pp.tile([128, 8, BC], F32, tag="xc_db")
            for c in range(8):
                ptr = ps_s.tile([128, 32], F32, tag="tr")
                nc.tensor.transpose(out=ptr[:, :BC],
                                    in_=xc_rows[:, 128 * c:128 * (c + 1)],
                                    identity=ident[:BC, :BC])
                nc.vector.tensor_copy(out=xc_db[:, c, :], in_=ptr[:, :BC])

            # ============ posterior logits ============
            ppo = ps_s.tile([1, 128], F32, tag="tr")
            for b in range(BC):
                for c in range(8):
                    nc.tensor.matmul(
                        ppo[:, 32 * b:32 * (b + 1)],
                        lhsT=xc_db[:, c, b:b + 1],
                        rhs=meank_dn[:, c, b, :],
                        start=(c == 0), stop=(c == 7))
            post_row = mp.tile([1, 128], F32, tag="prow")
            nc.vector.tensor_copy(out=post_row[:], in_=ppo[:])
            pod = dp.tile([BC, N], F32)
            nc.sync.dma_start(
                out=pod[:].rearrange("b n -> (b n)").rearrange("(o f) -> o f", o=1),
                in_=post_row[:])
            post_rows = mp.tile([BC, N], F32, tag="post_rows")
            nc.sync.dma_start(out=post_rows[:], in_=pod[:])

            # posterior = softmax(post_rows)
            mx2 = mp.tile([BC, 1], F32, tag="s1b")
            nc.vector.reduce_max(out=mx2[:], in_=post_rows[:], axis=AX)
            nmx2 = mp.tile([BC, 1], F32, tag="s2b")
            nc.scalar.mul(nmx2[:], mx2[:], -1.0)
            ex2 = mp.tile([BC, N], F32, tag="row6")
            sm2 = mp.tile([BC, 1], F32, tag="s3b")
            nc.scalar.activation(out=ex2[:], in_=post_rows[:], func=AF.Exp,
                                 bias=nmx2[:], scale=1.0, accum_out=sm2[:])
            rc2 = mp.tile([BC, 1], F32, tag="s4b")
            nc.vector.reciprocal(out=rc2[:], in_=sm2[:])
            po_out = mp.tile([BC, N], F32, tag="row7")
            nc.vector.tensor_scalar_mul(po_out[:], ex2[:], rc2[:])
            nc.sync.dma_start(out=post_o[:], in_=po_out[:])

            # K_index = softmax((post_logits + gumbel)/tau)
            gum_s = mp.tile([BC, N], F32, tag="gum_s")
            nc.sync.dma_start(out=gum_s[:], in_=gum[:])
            kg = mp.tile([BC, N], F32, tag="row8")
            nc.vector.tensor_add(out=kg[:], in0=post_rows[:], in1=gum_s[:])
            mx3 = mp.tile([BC, 1], F32, tag="s1c")
            nc.vector.reduce_max(out=mx3[:], in_=kg[:], axis=AX)
            nmx3 = mp.tile([BC, 1], F32, tag="s2c")
            nc.scalar.mul(nmx3[:], mx3[:], -1.0 / TAU)
            ex3 = mp.tile([BC, N], F32, tag="row9")
            sm3 = mp.tile([BC, 1], F32, tag="s3c")
            nc.scalar.activation(out=ex3[:], in_=kg[:], func=AF.Exp,
                                 bias=nmx3[:], scale=1.0 / TAU, accum_out=sm3[:])
            rc3 = mp.tile([BC, 1], F32, tag="s4c")
            nc.vector.reciprocal(out=rc3[:], in_=sm3[:])
            kidx_rows = pp.tile([BC, N], F32, tag="kidx_rows")
            nc.vector.tensor_scalar_mul(kidx_rows[:], ex3[:], rc3[:])
            nc.sync.dma_start(out=kidx_o[:], in_=kidx_rows[:])

            # ============ selected_K: gather argmax slice ============
            selw = mp.tile([BC, 1], F32, tag="selw")
            nc.vector.reduce_max(out=selw[:], in_=kidx_rows[:], axis=AX)
            ind = mp.tile([BC, N], F32, tag="row10")
            nc.vector.tensor_scalar(out=ind[:], in0=kidx_rows[:], scalar1=selw[:],
                                    scalar2=None, op0=ALU.is_equal)
            io32 = mp.tile([BC, N], F32, tag="io32")
            nc.sync.dma_start(out=io32[:], in_=_bcast_ap(iota32.ap(), BC, N))
            nc.vector.tensor_mul(out=ind[:], in0=ind[:], in1=io32[:])
            nsf = mp.tile([BC, 1], F32, tag="nsf")
            nc.vector.reduce_sum(out=nsf[:], in_=ind[:], axis=AX)
            # bounce [n*, w*] to DRAM for per-partition broadcasts
            nsw = dp.tile([BC, 2], F32)
            nc.sync.dma_start(out=nsw[:, 0:1], in_=nsf[:])
            nc.sync.dma_start(out=nsw[:, 1:2], in_=selw[:])
            nsw_ap = nsw[:]
            for b in range(BC):
                nsb = mp.tile([128, 1], F32, tag="nsb")
                nc.sync.dma_start(
                    out=nsb[:],
                    in_=bass.AP(tensor=nsw_ap.tensor, offset=nsw_ap.offset + 2 * b,
                                ap=[[0, 128], [1, 1]]))
                wsb = mp.tile([128, 1], F32, tag="wsb")
                nc.sync.dma_start(
                    out=wsb[:],
                    in_=bass.AP(tensor=nsw_ap.tensor, offset=nsw_ap.offset + 2 * b + 1,
                                ap=[[0, 128], [1, 1]]))
                idxf = mp.tile([128, 1], F32, tag="idxf")
                nc.scalar.mul(idxf[:], nsb[:], float(KS))
                nc.vector.tensor_add(out=idxf[:], in0=idxf[:], in1=iota128_s[:])
                idxw = mp.tile([128, 1], I32, tag="idxw")
                nc.vector.tensor_copy(out=idxw[:], in_=idxf[:])
                idxg = mp.tile([128, 1], F32, tag="idxg")
                nc.scalar.mul(idxg[:], nsb[:], 4.0)
                nc.vector.tensor_add(out=idxg[:], in0=idxg[:], in1=iotakh_s[:])
                nc.vector.tensor_scalar_add(idxg[:], idxg[:], float(b * N * KS))
                idxi = mp.tile([128, 1], I32, tag="idxi")
                nc.vector.tensor_copy(out=idxi[:], in_=idxg[:])
                selt = selp.tile([128, D], F32, tag="selt")
                nc.gpsimd.indirect_dma_start(
                    out=selt[:], out_offset=None,
                    in_=kh_flat,
                    in_offset=bass.IndirectOffsetOnAxis(ap=idxi[:, :1], axis=0))
                wmt = mp.tile([128, 1], F32, tag="wmt")
                nc.gpsimd.indirect_dma_start(
                    out=wmt[:], out_offset=None,
                    in_=wk_flat,
                    in_offset=bass.IndirectOffsetOnAxis(ap=idxw[:, :1], axis=0),
                    element_offset=b)
                wsel = mp.tile([128, 1], F32, tag="wsel")
                nc.vector.tensor_mul(out=wsel[:], in0=wmt[:], in1=wsb[:])
                nc.scalar.mul(wsel[:], wsel[:], float(KS))
                selo = selp.tile([128, D], F32, tag="selo")
                nc.vector.tensor_scalar_mul(selo[:], selt[:], wsel[:])
                nc.sync.dma_start(out=sel_o[b, :, :], in_=selo[:])

            # ============ vs = sum_n K_index * mean_K ; AllGather ============
            pkt = ps_s.tile([32, BC], F32, tag="tr")
            nc.tensor.transpose(out=pkt[:N, :BC], in_=kidx_rows[:], identity=ident[:BC, :BC])
            kidx_t = mp.tile([N, BC], F32, tag="kidx_t")
            nc.vector.tensor_copy(out=kidx_t[:], in_=pkt[:N, :BC])

            agi = dp.tile([BC, D], F32)
            ago = dp.tile([B, D], F32, addr_space="Shared")
            for b in range(BC):
                pvs = ps_a.tile([1, D], F32, tag="acc")
                for h in range(2):
                    nc.tensor.matmul(
                        pvs[:, 512 * h:512 * (h + 1)],
                        lhsT=kidx_t[:, b:b + 1],
                        rhs=meank_b[b][:, 512 * h:512 * (h + 1)],
                        start=True, stop=True)
                vsr = mp.tile([1, D], F32, tag="vsr")
                nc.vector.tensor_copy(out=vsr[:], in_=pvs[:])
                nc.sync.dma_start(out=agi[b:b + 1, :], in_=vsr[:])
            nc.gpsimd.collective_compute(
                "AllGather", ALU.bypass,
                replica_groups=[list(range(NC))],
                ins=[agi[:].opt()], outs=[ago[:].opt()])
            vs_all = pp.tile([B, D], F32, tag="vs_all")
            nc.sync.dma_start(out=vs_all[:], in_=ago[:])
            vs_db = pp.tile([128, 8, B], F32, tag="vs_db")
            for c in range(8):
                ptr = ps_s.tile([128, 32], F32, tag="tr")
                nc.tensor.transpose(out=ptr[:],
                                    in_=vs_all[:, 128 * c:128 * (c + 1)],
                                    identity=ident[:B, :B])
                nc.vector.tensor_copy(out=vs_db[:, c, :], in_=ptr[:])

            # ============ vocab projection + sharded log_softmax ============
            VH = VC // 2  # 2000
            logits = [pp.tile([B, VH], F32, tag=f"logits{vh}", name=f"logits{vh}")
                      for vh in range(2)]
            parts = []
            for vh in range(2):
                pv = ps_v.tile([B, 4, 512], F32, tag="voc")
                for dc in range(8):
                    w2tile = wp.tile([128, VH], F32, tag="w2tile")
                    nc.sync.dma_start(
                        out=w2tile[:], in_=w2t[128 * dc:128 * (dc + 1),
                                              VH * vh:VH * (vh + 1)])
                    for vt in range(4):
                        nc.tensor.matmul(
                            pv[:, vt, 0:500],
                            lhsT=vs_db[:, dc, :],
                            rhs=w2tile[:, 500 * vt:500 * (vt + 1)],
                            start=(dc == 0), stop=(dc == 7))
                b2b = bp.tile([B, VH], F32, tag="b2b")
                nc.sync.dma_start(out=b2b[:],
                                  in_=_bcast_ap(b2r.ap(), B, VH, offset=VH * vh))
                for vt in range(4):
                    nc.vector.tensor_add(
                        out=logits[vh][:, 500 * vt:500 * (vt + 1)],
                        in0=pv[:, vt, 0:500],
                        in1=b2b[:, 500 * vt:500 * (vt + 1)])
                exv = bp.tile([B, VH], F32, tag="exv")
                part = mp.tile([B, 1], F32, tag=f"part{vh}")
                nc.scalar.activation(out=exv[:], in_=logits[vh][:], func=AF.Exp,
                                     accum_out=part[:])
                parts.append(part)
            locsum = mp.tile([B, 1], F32, tag="locsum")
            nc.vector.tensor_add(out=locsum[:], in0=parts[0][:], in1=parts[1][:])
            ari = dp.tile([B, 1], F32)
            aro = dp.tile([B, 1], F32, addr_space="Shared")
            nc.sync.dma_start(out=ari[:], in_=locsum[:])
            nc.gpsimd.collective_compute(
                "AllReduce", ALU.add,
                replica_groups=[list(range(NC))],
                ins=[ari[:].opt()], outs=[aro[:].opt()])
            gsum = mp.tile([B, 1], F32, tag="gsum")
            nc.sync.dma_start(out=gsum[:], in_=aro[:])
            glog = mp.tile([B, 1], F32, tag="glog")
            nc.scalar.activation(out=glog[:], in_=gsum[:], func=AF.Ln)
            for vh in range(2):
                outv = bp.tile([B, VH], F32, tag="outv")
                nc.vector.tensor_scalar_sub(outv[:], logits[vh][:], glog[:])
                nc.sync.dma_start(out=vocab_o[:, VH * vh:VH * (vh + 1)], in_=outv[:])

    return nc


_NC_CACHE = None


def _get_program():
    global _NC_CACHE
    if _NC_CACHE is None:
        _NC_CACHE = build_program()
    return _NC_CACHE


def kernel(utterence_hidden, response_hidden, knowledge_hidden,
           utterence_pad_mask, response_pad_mask, knowledge_pad_mask,
           gumbel, W1, b1, W2, b2):
    global LAST_RESULTS
    f32 = np.float32
    uh = np.asarray(utterence_hidden, f32)
    rh = np.asarray(response_hidden, f32)
    kh = np.asarray(knowledge_hidden, f32)
    wu = ((~np.asarray(utterence_pad_mask, bool)).astype(f32) / XS)   # (B, XS)
    wr = ((~np.asarray(response_pad_mask, bool)).astype(f32) / YS)    # (B, YS)
    wk = ((~np.asarray(knowledge_pad_mask, bool)).astype(f32) / KS)   # (B, N, KS)
    gm = np.asarray(gumbel, f32)
    w1t = np.ascontiguousarray(np.asarray(W1, f32).T)                 # (2D, D)
    b1r = np.asarray(b1, f32).reshape(1, D)
    W2 = np.asarray(W2, f32)
    b2 = np.asarray(b2, f32)
    iota128 = np.arange(128, dtype=f32).reshape(128, 1)
    p = np.arange(128)
    iotakh = ((p // 4) * 128 + (p % 4)).astype(f32).reshape(128, 1)
    iota32 = np.arange(N, dtype=f32).reshape(1, N)

    in_maps = []
    for c in range(NC):
        bs = slice(BC * c, BC * (c + 1))
        bset = range(BC * c, BC * (c + 1))
        # block-diagonal lhsT for mean_X/mean_Y: (128, src, step, m)
        ubd = np.zeros((128, 2, 8, BC), f32)
        for src_i, w in enumerate((wu, wr)):
            for scn in range(2):
                for a in range(4):
                    step = scn * 4 + a
                    s0 = scn * 128 + 32 * a
                    for m, bg in enumerate(bset):
                        ubd[32 * m:32 * (m + 1), src_i, step, m] = w[bg, s0:s0 + 32]
        # block-diagonal lhsT for mean_K: (128, b, arr32, m32);
        # rows [4m:4m+4) of column m hold wk[bg, m, 4a:4a+4]
        kbd = np.zeros((128, BC, 32, 32), f32)
        midx = np.arange(32)
        for bl, bg in enumerate(bset):
            z = np.zeros((32, 4, 32, 32), f32)            # (m, j, a, m')
            z[midx, :, :, midx] = wk[bg].reshape(32, 32, 4).transpose(0, 2, 1)
            kbd[:, bl, :, :] = z.reshape(128, 32, 32)
        vsl = slice(VC * c, VC * (c + 1))
        in_maps.append({
            "u": np.ascontiguousarray(uh[:, bs, :]),
            "r": np.ascontiguousarray(rh[:, bs, :]),
            "kh": np.ascontiguousarray(kh[:, :, bs, :].transpose(2, 0, 1, 3).reshape(BC, N, 32, 4, D).transpose(0, 2, 1, 3, 4)),
            "ubd": ubd,
            "kbd": kbd,
            "wk_nsb": np.ascontiguousarray(wk[bs].transpose(1, 2, 0)),
            "gum": np.ascontiguousarray(gm[bs]),
            "w1t": w1t,
            "b1r": b1r,
            "w2t": np.ascontiguousarray(W2[vsl, :].T),
            "b2r": b2[vsl].reshape(1, VC),
            "iota128": iota128,
            "iotakh": iotakh,
            "iota32": iota32,
        })

    nc = _get_program()
    res = run_bass_kernel_spmd(nc, in_maps, core_ids=list(range(NC)),
                               trace=TRACE, **TRACE_KW)
    LAST_RESULTS = res
    rs = res.results
    prior = np.concatenate([rs[c]["prior_o"] for c in range(NC)], axis=0)
    posterior = np.concatenate([rs[c]["post_o"] for c in range(NC)], axis=0)
    k_index = np.concatenate([rs[c]["kidx_o"] for c in range(NC)], axis=0)
    selected = np.concatenate([rs[c]["sel_o"] for c in range(NC)], axis=0)
    vocab = np.concatenate([rs[c]["vocab_o"] for c in range(NC)], axis=1)
    return (prior, posterior, k_index, selected, vocab)
